# revision 20
# baseline (speedup 1.0000x reference)
"""Trainium2 Bass kernel for nn_Decoder (Bahdanau attention + LSTMCell decoder).

Sharding: data-parallel over batch B=64 across 8 NeuronCores (8 batches/core),
weights replicated, the 32-step scan fully local per core. No collectives.

Key structural choices (all matmuls bf16, fp32 PSUM accumulation):
  * dec-input fusion: dec_t = h_t @ fc_w.T + fc_b is folded into the gate
    recurrence (W_comb = w_hh + w_ih[:, :OUT] @ fc_w), so the fc output is
    computed off the critical chain; step 0 uses the original weights
    (dec_in(0) = 0).
  * softmax without max-subtraction (energies are bounded, |e| < ~4).
  * context via col-tiled matmuls: 4 concurrent PE column-strips, lhsT are
    zero-padded per-(batch, s-chunk) softmax-weight tiles; 1/sum folded into
    the PSUM evacuation scale.
  * all [row, feature] -> [feature-partition, batch] transposes are PE
    128x128 transposes + one strided DVE copy each (engines only ever touch
    a single partition window per op; PE-transpose is the cross-window mover).
  * LSTM elementwise runs in transposed space [h-partition, batch-free]
    (tiny free dims); c stays fp32.
"""
import os
from contextlib import ExitStack

import numpy as np
import ml_dtypes

import concourse.bass as bass
import concourse.tile as tile
from concourse import bacc, mybir
from concourse._compat import with_exitstack
from concourse.bass_utils import run_bass_kernel_spmd

F32 = mybir.dt.float32
BF16 = mybir.dt.bfloat16
OP = mybir.AluOpType
ACTF = mybir.ActivationFunctionType
AX = mybir.AxisListType

B, S, H, OUT, STEPS = 64, 1024, 512, 256, 32
NCORES = 8
BL = B // NCORES          # 8 local batches
SO = S // 128             # 8 s-chunks
HC = H // 128             # 4 h-chunks
G4 = 4 * H                # 2048

BF = ml_dtypes.bfloat16
DEV_STEPS = int(os.environ.get("KERNEL_STEPS", STEPS))

IN_SPECS = [
    ("enc_sb", [128, BL * SO * H], "BF16"),
    ("enc_energy", [128, BL * SO], "F32"),
    ("w_ihcT", [128, HC * G4], "BF16"),
    ("w_hhT0", [128, HC * G4], "BF16"),
    ("w_cmbT", [128, HC * G4], "BF16"),
    ("fc_wT", [128, HC * OUT], "BF16"),
    ("wa_bc", [128, HC * 128], "BF16"),
    ("bias_g0", [1, G4], "BF16"),
    ("bias_gc", [1, G4], "BF16"),
    ("bias_fc", [1, OUT], "BF16"),
    ("h0T", [128, HC * 32], "BF16"),
    ("ident", [128, 128], "BF16"),
]


@with_exitstack
def decoder_kernel(ctx: ExitStack, tc: tile.TileContext, io: dict):
    nc = tc.nc
    P = 128

    const = ctx.enter_context(tc.tile_pool(name="const", bufs=1))
    state = ctx.enter_context(tc.tile_pool(name="state", bufs=1))
    tmp = ctx.enter_context(tc.tile_pool(name="tmp", bufs=2))
    decp = ctx.enter_context(tc.tile_pool(name="decp", bufs=2))
    psum = ctx.enter_context(tc.tile_pool(name="psum", bufs=1, space="PSUM"))
    psumT = ctx.enter_context(tc.tile_pool(name="psumT", bufs=1, space="PSUM"))
    psumD = ctx.enter_context(tc.tile_pool(name="psumD", bufs=1, space="PSUM"))

    # ---------------- constants ----------------
    ones1 = const.tile([1, 8], BF16)
    nc.vector.memset(ones1[:], 1.0)
    onesc = const.tile([P, 1], BF16)
    nc.vector.memset(onesc[:], 1.0)
    tiles = {}
    for name, shape, dts in IN_SPECS:
        dt = BF16 if dts == "BF16" else F32
        t_ = const.tile(shape, dt, tag=name)
        nc.sync.dma_start(t_[:], io[name][:])
        tiles[name] = t_

    encv = tiles["enc_sb"][:].rearrange("p (b so h) -> p b so h", b=BL, so=SO, h=H)
    enc_e = tiles["enc_energy"]
    w_ihcTv = tiles["w_ihcT"][:].rearrange("p (k j) -> p k j", k=HC, j=G4)
    w_hhT0v = tiles["w_hhT0"][:].rearrange("p (k j) -> p k j", k=HC, j=G4)
    w_cmbTv = tiles["w_cmbT"][:].rearrange("p (k j) -> p k j", k=HC, j=G4)
    fc_wTv = tiles["fc_wT"][:].rearrange("p (k o) -> p k o", k=HC, o=OUT)
    wa_bcv = tiles["wa_bc"][:].rearrange("p (k m) -> p k m", k=HC, m=P)
    ident = tiles["ident"]

    # ---------------- state ----------------
    hT = state.tile([P, HC * 32], BF16)               # [p, (kc, b32)]
    nc.sync.dma_start(hT[:], io["h0T"])
    hTv = hT[:].rearrange("p (k b) -> p k b", k=HC, b=32)

    cT = state.tile([P, HC * 8], F32)                 # [p, (kc, b8)]
    nc.vector.memset(cT[:], 0.0)
    cTv = cT[:].rearrange("p (k b) -> p k b", k=HC, b=8)

    xT_pad = state.tile([P, HC * 8], BF16)            # ctx.T dense [p, (hq, b8)]

    Z = state.tile([P, 528], BF16)                    # zero-padded exp lhsT slots
    nc.vector.memset(Z[:], 0.0)
    Zj = Z[:].rearrange("p (j r) -> p j r", j=4, r=132)

    sums_pad = state.tile([1, 8], F32)
    recip_pad = state.tile([32, 40], F32)
    nc.vector.memset(recip_pad[:], 0.0)
    recip_sp = state.tile([P, 32], F32)

    ctx_bf = state.tile([P, 512], BF16)               # spread rows {32j+bm}
    nc.vector.memset(ctx_bf[:], 0.0)
    gact = state.tile([P, 512], BF16)                 # spread rows {32j2+b}
    nc.vector.memset(gact[:], 0.0)
    gT = state.tile([P, HC * 4 * 8], BF16)            # [p, (hq, gate, b8)]
    gTv = gT[:].rearrange("p (q g b) -> p q g b", q=HC, g=4, b=8)

    # ---------------- psum ----------------
    ps_strip = []
    for j in range(4):
        pt = psum.tile([P, 512], F32, tag=f"ps_strip{j}")
        nc.vector.memset(pt[:], 0.0)
        ps_strip.append(pt)
    ps_E = psum.tile([P, 8], F32, tag="ps_E")
    nc.vector.memset(ps_E[:], 0.0)
    ps_S = psum.tile([1, 512], F32, tag="ps_S")

    out_dram = io["out_dec"]

    for t in range(DEV_STEPS):
        # ===== A: energy addend =====
        for kc in range(HC):
            nc.tensor.matmul(
                ps_E[:, 0:8], wa_bcv[:, kc, :], hTv[:, kc, 0:8],
                start=(kc == 0), stop=(kc == HC - 1),
            )
        energy = tmp.tile([P, BL * SO], F32, tag="energy")
        eb = ps_E[:, 0:8].rearrange("p (b one) -> p b one", one=1).broadcast_to((P, BL, SO))
        nc.vector.tensor_tensor(
            energy[:].rearrange("p (b so) -> p b so", b=BL, so=SO),
            enc_e[:].rearrange("p (b so) -> p b so", b=BL, so=SO),
            eb, OP.add,
        )
        # ===== exp into Z slots (idx = 132j + 66bm + 8so + bm) =====
        ev = energy[:].rearrange("p (j bm so) -> p j bm so", j=4, bm=2, so=SO)
        for bm in range(2):
            zslice = Zj[:, :, 66 * bm + bm : 66 * bm + bm + 64].rearrange(
                "p j (so e) -> p j so e", so=SO, e=8
            )[:, :, :, 0]
            nc.scalar.activation(zslice, ev[:, :, bm, :], ACTF.Exp)
        # ===== per-batch sums -> recip, spread =====
        ps_sums = ps_S
        for bm in range(2):
            for j in range(4):
                sl = (bm * 4 + j) * 64
                nc.tensor.matmul(
                    ps_sums[0:1, sl : sl + 64],
                    onesc[:, :], Zj[:, j, 66 * bm : 66 * bm + 64],
                    start=True, stop=True,
                )
        sv = ps_sums[0:1, :].rearrange("o (bm j q) -> o bm j q", bm=2, j=4, q=64)
        so_out = sums_pad[0:1, 0:8].rearrange("o (j bm) -> o bm j", j=4, bm=2)
        nc.vector.tensor_reduce(so_out, sv, AX.X, OP.add)
        nc.vector.reciprocal(recip_pad[0:1, 0:8], sums_pad[0:1, 0:8])
        for j in range(4):
            nc.vector.transpose(
                recip_sp[32 * j : 32 * j + 32, :], recip_pad[0:32, 2 * j : 2 * j + 32]
            )

        # ===== context matmuls (col-tiled) =====
        for so in range(SO):
            for bm in range(2):
                for j in range(4):
                    b = 2 * j + bm
                    base = 132 * j + 66 * bm + 8 * so
                    nc.tensor.matmul(
                        ps_strip[j][32 * j : 32 * j + 8, :],
                        Z[:, base : base + 8], encv[:, b, so, :],
                        start=(so == 0 and bm == 0), stop=(so == SO - 1 and bm == 1),
                        tile_position=(0, 32 * j),
                    )
        # evacuate + normalize (same-window)
        for j in range(4):
            w = slice(32 * j, 32 * j + 2)
            if j % 2 == 0:
                nc.scalar.activation(
                    ctx_bf[w, :], ps_strip[j][w, :], ACTF.Copy, scale=recip_sp[w, 0:1]
                )
            else:
                nc.vector.tensor_scalar_mul(ctx_bf[w, :], ps_strip[j][w, :], recip_sp[w, 0:1])
        # ctx transpose: PE 128x128 + strided copy -> xT_pad
        for hq in range(HC):
            trT = psumT.tile([P, P], BF16, tag="trT")
            nc.tensor.transpose(trT[:], ctx_bf[:, hq * 128 : (hq + 1) * 128], ident[:])
            src = trT[:].rearrange("p (j r) -> p j r", j=4, r=32)[:, :, 0:2]
            nc.vector.tensor_copy(
                xT_pad[:, hq * 8 : hq * 8 + 8].rearrange("p (j b) -> p j b", j=4, b=2), src
            )

        # ===== gates (col-tiled; strip j2 = gate j2: order i,f,g,o) =====
        whT = w_hhT0v if t == 0 else w_cmbTv
        bias_t = tiles["bias_g0"] if t == 0 else tiles["bias_gc"]
        for j2 in range(4):
            nc.tensor.matmul(
                ps_strip[j2][32 * j2 : 32 * j2 + 8, :],
                ones1[:, :], bias_t[:, 512 * j2 : 512 * (j2 + 1)],
                start=True, stop=False, tile_position=(0, 32 * j2),
            )
        for hq in range(HC):
            for j2 in range(4):
                nc.tensor.matmul(
                    ps_strip[j2][32 * j2 : 32 * j2 + 8, :],
                    xT_pad[:, hq * 8 : hq * 8 + 8],
                    w_ihcTv[:, hq, 512 * j2 : 512 * (j2 + 1)],
                    start=False, stop=False, tile_position=(0, 32 * j2),
                )
        for kc in range(HC):
            for j2 in range(4):
                nc.tensor.matmul(
                    ps_strip[j2][32 * j2 : 32 * j2 + 8, :],
                    hTv[:, kc, 0:8],
                    whT[:, kc, 512 * j2 : 512 * (j2 + 1)],
                    start=False, stop=(kc == HC - 1), tile_position=(0, 32 * j2),
                )
        # nonlinearities (same-window), bf16
        for j2 in range(4):
            w = slice(32 * j2, 32 * j2 + 8)
            fn = ACTF.Tanh if j2 == 2 else ACTF.Sigmoid
            nc.scalar.activation(gact[w, :], ps_strip[j2][w, :], fn)
        # gate transpose: PE 128x128 + strided copy -> gT
        for hq in range(HC):
            trT = psumT.tile([P, P], BF16, tag="trT")
            nc.tensor.transpose(trT[:], gact[:, hq * 128 : (hq + 1) * 128], ident[:])
            src = trT[:].rearrange("p (g r) -> p g r", g=4, r=32)[:, :, 0:8]
            nc.vector.tensor_copy(gTv[:, hq, :, :], src)

        # ===== elementwise (transposed space) =====
        tmp_ig = tmp.tile([P, HC * 8], F32, tag="tmp_ig")
        tigv = tmp_ig[:].rearrange("p (k b) -> p k b", k=HC, b=8)
        nc.vector.tensor_tensor(tigv, gTv[:, :, 0, :], gTv[:, :, 2, :], OP.mult)
        nc.vector.tensor_tensor(cTv, cTv, gTv[:, :, 1, :], OP.mult)
        nc.vector.tensor_tensor(cTv, cTv, tigv, OP.add)
        tanh_c = tmp.tile([P, HC * 8], BF16, tag="tanh_c")
        tcv = tanh_c[:].rearrange("p (k b) -> p k b", k=HC, b=8)
        nc.scalar.activation(tcv, cTv, ACTF.Tanh)
        nc.vector.tensor_tensor(hTv[:, :, 0:8], gTv[:, :, 3, :], tcv, OP.mult)

        # ===== dec output (off the critical chain) =====
        ps_dec = psumD.tile([32, OUT], F32, tag="ps_dec")
        nc.tensor.matmul(ps_dec[0:8, :], ones1[:, :], tiles["bias_fc"][:, :], start=True, stop=False)
        for kc in range(HC):
            nc.tensor.matmul(
                ps_dec[0:8, :], hTv[:, kc, 0:8], fc_wTv[:, kc, :],
                start=False, stop=(kc == HC - 1),
            )
        dec_out = decp.tile([8, OUT], F32, tag="dec_out")
        nc.scalar.activation(dec_out[:], ps_dec[0:8, :], ACTF.Copy)
        nc.sync.dma_start(out_dram[:, t, :], dec_out[:])

        if t == 0 and "dbg_energy" in io:
            nc.sync.dma_start(io["dbg_energy"], energy[:])
            zf = tmp.tile([P, 528], F32, tag="zf")
            nc.vector.tensor_copy(zf[:], Z[:])
            nc.sync.dma_start(io["dbg_Z"], zf[:])
            cf = tmp.tile([P, 512], F32, tag="cf")
            nc.vector.tensor_copy(cf[:], ctx_bf[:])
            nc.sync.dma_start(io["dbg_ctx"], cf[:])
            xf = tmp.tile([P, HC * 8], F32, tag="xf")
            nc.vector.tensor_copy(xf[:], xT_pad[:])
            nc.sync.dma_start(io["dbg_xtpad"], xf[:])
            gf = tmp.tile([P, 512], F32, tag="gf")
            nc.vector.tensor_copy(gf[:], gact[:])
            nc.sync.dma_start(io["dbg_gact"], gf[:])
            hf = tmp.tile([P, HC * 32], F32, tag="hf")
            nc.vector.tensor_copy(hf[:], hT[:])
            nc.sync.dma_start(io["dbg_hT"], hf[:])
            rf = tmp.tile([P, 1], F32, tag="rf")
            nc.vector.tensor_copy(rf[:], recip_sp[:, 0:1])
            nc.sync.dma_start(io["dbg_recip"], rf[:])
            sf = tmp.tile([1, 8], F32, tag="sf")
            nc.vector.tensor_copy(sf[:], sums_pad[:])
            nc.sync.dma_start(io["dbg_sums"], sf[:])
            rp = tmp.tile([32, 40], F32, tag="rp")
            nc.vector.tensor_copy(rp[:], recip_pad[:])
            nc.sync.dma_start(io["dbg_rpad"], rp[:])


# ---------------------------------------------------------------------------
# Host driver
# ---------------------------------------------------------------------------
_CACHE = {}


def _build():
    if "nc" in _CACHE:
        return _CACHE["nc"]
    nc = bacc.Bacc("TRN2", target_bir_lowering=False, debug=False, num_devices=NCORES)
    io = {}
    for name, shape, dts in IN_SPECS:
        io[name] = nc.dram_tensor(name, shape, BF16 if dts == "BF16" else F32, kind="ExternalInput").ap()
    io["out_dec"] = nc.dram_tensor("out_dec", [BL, STEPS, OUT], F32, kind="ExternalOutput").ap()
    with tile.TileContext(nc) as tc:
        decoder_kernel(tc, io)
    nc.compile()
    _CACHE["nc"] = nc
    return nc


def _chunked(w):
    """[k, j] -> [128, (kc, j)] with k = kc*128 + p."""
    k, j = w.shape
    return np.ascontiguousarray(w.reshape(k // 128, 128, j).transpose(1, 0, 2).reshape(128, -1))


def _prep_core(enc_l, h_l, attn_w, attn_b, w_ih, w_hh, b_ih, b_hh, fc_w, fc_b):
    wa_e, wa_d = attn_w[:H], attn_w[H:]
    enc_sb = np.ascontiguousarray(
        enc_l.reshape(BL, SO, 128, H).transpose(2, 0, 1, 3).reshape(128, -1)
    ).astype(BF)
    ee = enc_l @ wa_e + attn_b[0]
    enc_energy = np.ascontiguousarray(
        ee.reshape(BL, SO, 128).transpose(2, 0, 1).reshape(128, -1)
    ).astype(np.float32)

    w_d = w_ih[:, :OUT]                                   # dec-input part [2048, 256]
    w_c = w_ih[:, OUT:]                                   # ctx part [2048, 512]
    w_cmb = w_hh + w_d @ fc_w                             # [2048, 512]
    bias0 = b_ih + b_hh
    biasc = bias0 + w_d @ fc_b

    h0T = np.zeros((128, HC, 32), dtype=BF)
    h0T[:, :, :BL] = h_l.T.reshape(HC, 128, BL).transpose(1, 0, 2).astype(BF)
    return {
        "enc_sb": enc_sb,
        "enc_energy": enc_energy,
        "w_ihcT": _chunked(w_c.T).astype(BF),
        "w_hhT0": _chunked(w_hh.T).astype(BF),
        "w_cmbT": _chunked(w_cmb.T).astype(BF),
        "fc_wT": _chunked(fc_w.T).astype(BF),
        "wa_bc": np.ascontiguousarray(
            np.broadcast_to(wa_d.reshape(HC, 128, 1), (HC, 128, 128)).transpose(1, 0, 2).reshape(128, -1)
        ).astype(BF),
        "bias_g0": bias0.reshape(1, G4).astype(BF),
        "bias_gc": biasc.reshape(1, G4).astype(BF),
        "bias_fc": fc_b.reshape(1, OUT).astype(BF),
        "h0T": h0T.reshape(128, -1),
        "ident": np.eye(128, dtype=np.float32).astype(BF),
    }


def kernel(encoder_outputs, hidden, attn_w, attn_b, w_ih, w_hh, b_ih, b_hh, fc_w, fc_b):
    encoder_outputs = np.asarray(encoder_outputs, dtype=np.float32)
    hidden = np.asarray(hidden, dtype=np.float32)
    args = [np.asarray(a, dtype=np.float32) for a in (attn_w, attn_b, w_ih, w_hh, b_ih, b_hh, fc_w, fc_b)]

    nc = _build()
    in_maps = []
    for cidx in range(NCORES):
        sl = slice(cidx * BL, (cidx + 1) * BL)
        in_maps.append(_prep_core(encoder_outputs[sl], hidden[sl], *args))
    res = run_bass_kernel_spmd(nc, in_maps, list(range(NCORES)))
    outs = [res.results[cidx]["out_dec"] for cidx in range(NCORES)]
    return np.concatenate(outs, axis=0)


# revision 22
# speedup vs baseline: 14.8436x; 14.8436x over previous
"""Trainium2 Bass kernel for nn_Decoder (Bahdanau attention + LSTMCell decoder).

Sharding: data-parallel over batch B=64 across 8 NeuronCores (8 batches/core),
weights replicated, the 32-step scan fully local per core. No collectives.

Key structural choices (all matmuls bf16, fp32 PSUM accumulation):
  * dec-input fusion: dec_t = h_t @ fc_w.T + fc_b is folded into the gate
    recurrence (W_comb = w_hh + w_ih[:, :OUT] @ fc_w), so the fc output is
    computed off the critical chain; step 0 uses the original weights
    (dec_in(0) = 0).
  * softmax without max-subtraction (energies are bounded, |e| < ~4).
  * context via col-tiled matmuls: 4 concurrent PE column-strips, lhsT are
    zero-padded per-(batch, s-chunk) softmax-weight tiles; 1/sum folded into
    the PSUM evacuation scale.
  * all [row, feature] -> [feature-partition, batch] transposes are PE
    128x128 transposes + one strided DVE copy each (engines only ever touch
    a single partition window per op; PE-transpose is the cross-window mover).
  * LSTM elementwise runs in transposed space [h-partition, batch-free]
    (tiny free dims); c stays fp32.
"""
import os
from contextlib import ExitStack

import numpy as np
import ml_dtypes

import concourse.bass as bass
import concourse.tile as tile
from concourse import bacc, mybir
from concourse._compat import with_exitstack
from concourse.bass_utils import run_bass_kernel_spmd

F32 = mybir.dt.float32
BF16 = mybir.dt.bfloat16
OP = mybir.AluOpType
ACTF = mybir.ActivationFunctionType
AX = mybir.AxisListType

B, S, H, OUT, STEPS = 64, 1024, 512, 256, 32
NCORES = 8
BL = B // NCORES          # 8 local batches
SO = S // 128             # 8 s-chunks
HC = H // 128             # 4 h-chunks
G4 = 4 * H                # 2048

BF = ml_dtypes.bfloat16
DEV_STEPS = int(os.environ.get("KERNEL_STEPS", STEPS))

IN_SPECS = [
    ("enc_sb", [128, BL * SO * H], "BF16"),
    ("enc_energy", [128, BL * SO], "F32"),
    ("w_ihcT", [128, HC * G4], "BF16"),
    ("w_hhT0", [128, HC * G4], "BF16"),
    ("w_cmbT", [128, HC * G4], "BF16"),
    ("fc_wT", [128, HC * OUT], "BF16"),
    ("wa_bc", [128, HC * 128], "BF16"),
    ("bias_g0", [1, G4], "BF16"),
    ("bias_gc", [1, G4], "BF16"),
    ("bias_fc", [1, OUT], "BF16"),
    ("h0T", [128, HC * 32], "BF16"),
    ("ident", [128, 128], "BF16"),
]


@with_exitstack
def decoder_kernel(ctx: ExitStack, tc: tile.TileContext, io: dict):
    nc = tc.nc
    P = 128

    const = ctx.enter_context(tc.tile_pool(name="const", bufs=1))
    state = ctx.enter_context(tc.tile_pool(name="state", bufs=1))
    tmp = ctx.enter_context(tc.tile_pool(name="tmp", bufs=2))
    decp = ctx.enter_context(tc.tile_pool(name="decp", bufs=2))
    psum = ctx.enter_context(tc.tile_pool(name="psum", bufs=1, space="PSUM"))
    psumT = ctx.enter_context(tc.tile_pool(name="psumT", bufs=1, space="PSUM"))
    psumD = ctx.enter_context(tc.tile_pool(name="psumD", bufs=1, space="PSUM"))

    # ---------------- constants ----------------
    ones1 = const.tile([1, 8], BF16)
    nc.vector.memset(ones1[:], 1.0)
    onesc = const.tile([P, 1], BF16)
    nc.vector.memset(onesc[:], 1.0)
    tiles = {}
    for name, shape, dts in IN_SPECS:
        dt = BF16 if dts == "BF16" else F32
        t_ = const.tile(shape, dt, tag=name)
        nc.sync.dma_start(t_[:], io[name][:])
        tiles[name] = t_

    encv = tiles["enc_sb"][:].rearrange("p (b so h) -> p b so h", b=BL, so=SO, h=H)
    enc_e = tiles["enc_energy"]
    w_ihcTv = tiles["w_ihcT"][:].rearrange("p (k j) -> p k j", k=HC, j=G4)
    w_hhT0v = tiles["w_hhT0"][:].rearrange("p (k j) -> p k j", k=HC, j=G4)
    w_cmbTv = tiles["w_cmbT"][:].rearrange("p (k j) -> p k j", k=HC, j=G4)
    fc_wTv = tiles["fc_wT"][:].rearrange("p (k o) -> p k o", k=HC, o=OUT)
    wa_bcv = tiles["wa_bc"][:].rearrange("p (k m) -> p k m", k=HC, m=P)
    ident = tiles["ident"]

    # ---------------- state ----------------
    hT = state.tile([P, HC * 32], BF16)               # [p, (kc, b32)]
    nc.sync.dma_start(hT[:], io["h0T"])
    hTv = hT[:].rearrange("p (k b) -> p k b", k=HC, b=32)

    cT = state.tile([P, HC * 8], F32)                 # [p, (kc, b8)]
    nc.vector.memset(cT[:], 0.0)
    cTv = cT[:].rearrange("p (k b) -> p k b", k=HC, b=8)

    xT_pad = state.tile([P, HC * 8], BF16)            # ctx.T dense [p, (hq, b8)]

    Z = state.tile([P, 528], BF16)                    # zero-padded exp lhsT slots
    nc.vector.memset(Z[:], 0.0)
    Zj = Z[:].rearrange("p (j r) -> p j r", j=4, r=132)

    sums_pad = state.tile([1, 8], F32)
    recip_pad = state.tile([32, 40], F32)
    nc.vector.memset(recip_pad[:], 0.0)
    recip_sp = state.tile([P, 32], F32)

    ctx_bf = state.tile([P, 512], BF16)               # spread rows {32j+bm}
    nc.vector.memset(ctx_bf[:], 0.0)
    gact = state.tile([P, 512], BF16)                 # spread rows {32j2+b}
    nc.vector.memset(gact[:], 0.0)
    gT = state.tile([P, HC * 4 * 8], BF16)            # [p, (hq, gate, b8)]
    gTv = gT[:].rearrange("p (q g b) -> p q g b", q=HC, g=4, b=8)

    # ---------------- psum ----------------
    ps_strip = []
    for j in range(4):
        pt = psum.tile([P, 512], F32, tag=f"ps_strip{j}")
        nc.vector.memset(pt[:], 0.0)
        ps_strip.append(pt)
    ps_E = psum.tile([P, 8], F32, tag="ps_E")
    nc.vector.memset(ps_E[:], 0.0)
    ps_S = psum.tile([1, 512], F32, tag="ps_S")

    out_dram = io["out_dec"]

    for t in range(DEV_STEPS):
        # ===== A: energy addend =====
        for kc in range(HC):
            nc.tensor.matmul(
                ps_E[:, 0:8], wa_bcv[:, kc, :], hTv[:, kc, 0:8],
                start=(kc == 0), stop=(kc == HC - 1),
            )
        energy = tmp.tile([P, BL * SO], F32, tag="energy")
        eb = ps_E[:, 0:8].rearrange("p (b one) -> p b one", one=1).broadcast_to((P, BL, SO))
        nc.vector.tensor_tensor(
            energy[:].rearrange("p (b so) -> p b so", b=BL, so=SO),
            enc_e[:].rearrange("p (b so) -> p b so", b=BL, so=SO),
            eb, OP.add,
        )
        # ===== exp into Z slots (idx = 132j + 66bm + 8so + bm) =====
        ev = energy[:].rearrange("p (j bm so) -> p j bm so", j=4, bm=2, so=SO)
        for bm in range(2):
            zslice = Zj[:, :, 66 * bm + bm : 66 * bm + bm + 64].rearrange(
                "p j (so e) -> p j so e", so=SO, e=8
            )[:, :, :, 0]
            nc.scalar.activation(zslice, ev[:, :, bm, :], ACTF.Exp)
        # ===== per-batch sums -> recip, spread =====
        ps_sums = ps_S
        for bm in range(2):
            for j in range(4):
                sl = (bm * 4 + j) * 64
                nc.tensor.matmul(
                    ps_sums[0:1, sl : sl + 64],
                    onesc[:, :], Zj[:, j, 66 * bm : 66 * bm + 64],
                    start=True, stop=True,
                )
        sv = ps_sums[0:1, :].rearrange("o (bm j q) -> o bm j q", bm=2, j=4, q=64)
        so_out = sums_pad[0:1, 0:8].rearrange("o (j bm) -> o bm j", j=4, bm=2)
        nc.vector.tensor_reduce(so_out, sv, AX.X, OP.add)
        nc.vector.reciprocal(recip_pad[0:1, 0:8], sums_pad[0:1, 0:8])
        for j in range(4):
            nc.vector.transpose(
                recip_sp[32 * j : 32 * j + 32, :], recip_pad[0:32, 2 * j : 2 * j + 32]
            )

        # ===== context matmuls (col-tiled) =====
        for so in range(SO):
            for bm in range(2):
                for j in range(4):
                    b = 2 * j + bm
                    base = 132 * j + 66 * bm + 8 * so
                    nc.tensor.matmul(
                        ps_strip[j][32 * j : 32 * j + 8, :],
                        Z[:, base : base + 8], encv[:, b, so, :],
                        start=(so == 0 and bm == 0), stop=(so == SO - 1 and bm == 1),
                        tile_position=(0, 32 * j),
                    )
        # evacuate + normalize (same-window)
        for j in range(4):
            w = slice(32 * j, 32 * j + 2)
            if j % 2 == 0:
                nc.scalar.activation(
                    ctx_bf[w, :], ps_strip[j][w, :], ACTF.Copy, scale=recip_sp[w, 0:1]
                )
            else:
                nc.vector.tensor_scalar_mul(ctx_bf[w, :], ps_strip[j][w, :], recip_sp[w, 0:1])
        # ctx transpose: PE 128x128 + strided copy -> xT_pad
        for hq in range(HC):
            trT = psumT.tile([P, P], BF16, tag="trT")
            nc.tensor.transpose(trT[:], ctx_bf[:, hq * 128 : (hq + 1) * 128], ident[:])
            src = trT[:].rearrange("p (j r) -> p j r", j=4, r=32)[:, :, 0:2]
            nc.vector.tensor_copy(
                xT_pad[:, hq * 8 : hq * 8 + 8].rearrange("p (j b) -> p j b", j=4, b=2), src
            )

        # ===== gates (col-tiled; strip j2 = gate j2: order i,f,g,o) =====
        whT = w_hhT0v if t == 0 else w_cmbTv
        bias_t = tiles["bias_g0"] if t == 0 else tiles["bias_gc"]
        for j2 in range(4):
            nc.tensor.matmul(
                ps_strip[j2][32 * j2 : 32 * j2 + 8, :],
                ones1[:, :], bias_t[:, 512 * j2 : 512 * (j2 + 1)],
                start=True, stop=False, tile_position=(0, 32 * j2),
            )
        for hq in range(HC):
            for j2 in range(4):
                nc.tensor.matmul(
                    ps_strip[j2][32 * j2 : 32 * j2 + 8, :],
                    xT_pad[:, hq * 8 : hq * 8 + 8],
                    w_ihcTv[:, hq, 512 * j2 : 512 * (j2 + 1)],
                    start=False, stop=False, tile_position=(0, 32 * j2),
                )
        for kc in range(HC):
            for j2 in range(4):
                nc.tensor.matmul(
                    ps_strip[j2][32 * j2 : 32 * j2 + 8, :],
                    hTv[:, kc, 0:8],
                    whT[:, kc, 512 * j2 : 512 * (j2 + 1)],
                    start=False, stop=(kc == HC - 1), tile_position=(0, 32 * j2),
                )
        # nonlinearities (same-window), bf16
        for j2 in range(4):
            w = slice(32 * j2, 32 * j2 + 8)
            fn = ACTF.Tanh if j2 == 2 else ACTF.Sigmoid
            nc.scalar.activation(gact[w, :], ps_strip[j2][w, :], fn)
        # gate transpose: PE 128x128 + strided copy -> gT
        for hq in range(HC):
            trT = psumT.tile([P, P], BF16, tag="trT")
            nc.tensor.transpose(trT[:], gact[:, hq * 128 : (hq + 1) * 128], ident[:])
            src = trT[:].rearrange("p (g r) -> p g r", g=4, r=32)[:, :, 0:8]
            nc.vector.tensor_copy(gTv[:, hq, :, :], src)

        # ===== elementwise (transposed space) =====
        tmp_ig = tmp.tile([P, HC * 8], F32, tag="tmp_ig")
        tigv = tmp_ig[:].rearrange("p (k b) -> p k b", k=HC, b=8)
        nc.vector.tensor_tensor(tigv, gTv[:, :, 0, :], gTv[:, :, 2, :], OP.mult)
        nc.vector.tensor_tensor(cTv, cTv, gTv[:, :, 1, :], OP.mult)
        nc.vector.tensor_tensor(cTv, cTv, tigv, OP.add)
        tanh_c = tmp.tile([P, HC * 8], BF16, tag="tanh_c")
        tcv = tanh_c[:].rearrange("p (k b) -> p k b", k=HC, b=8)
        nc.scalar.activation(tcv, cTv, ACTF.Tanh)
        nc.vector.tensor_tensor(hTv[:, :, 0:8], gTv[:, :, 3, :], tcv, OP.mult)

        # ===== dec output (off the critical chain) =====
        ps_dec = psumD.tile([32, OUT], F32, tag="ps_dec")
        nc.tensor.matmul(ps_dec[0:8, :], ones1[:, :], tiles["bias_fc"][:, :], start=True, stop=False)
        for kc in range(HC):
            nc.tensor.matmul(
                ps_dec[0:8, :], hTv[:, kc, 0:8], fc_wTv[:, kc, :],
                start=False, stop=(kc == HC - 1),
            )
        dec_out = decp.tile([8, OUT], F32, tag="dec_out")
        nc.scalar.activation(dec_out[:], ps_dec[0:8, :], ACTF.Copy)
        nc.sync.dma_start(out_dram[:, t, :], dec_out[:])

        if t == 0 and "dbg_energy" in io:
            nc.sync.dma_start(io["dbg_energy"], energy[:])
            zf = tmp.tile([P, 528], F32, tag="zf")
            nc.vector.tensor_copy(zf[:], Z[:])
            nc.sync.dma_start(io["dbg_Z"], zf[:])
            cf = tmp.tile([P, 512], F32, tag="cf")
            nc.vector.tensor_copy(cf[:], ctx_bf[:])
            nc.sync.dma_start(io["dbg_ctx"], cf[:])
            xf = tmp.tile([P, HC * 8], F32, tag="xf")
            nc.vector.tensor_copy(xf[:], xT_pad[:])
            nc.sync.dma_start(io["dbg_xtpad"], xf[:])
            gf = tmp.tile([P, 512], F32, tag="gf")
            nc.vector.tensor_copy(gf[:], gact[:])
            nc.sync.dma_start(io["dbg_gact"], gf[:])
            hf = tmp.tile([P, HC * 32], F32, tag="hf")
            nc.vector.tensor_copy(hf[:], hT[:])
            nc.sync.dma_start(io["dbg_hT"], hf[:])
            rf = tmp.tile([P, 1], F32, tag="rf")
            nc.vector.tensor_copy(rf[:], recip_sp[:, 0:1])
            nc.sync.dma_start(io["dbg_recip"], rf[:])
            sf = tmp.tile([1, 8], F32, tag="sf")
            nc.vector.tensor_copy(sf[:], sums_pad[:])
            nc.sync.dma_start(io["dbg_sums"], sf[:])
            rp = tmp.tile([32, 40], F32, tag="rp")
            nc.vector.tensor_copy(rp[:], recip_pad[:])
            nc.sync.dma_start(io["dbg_rpad"], rp[:])


# ---------------------------------------------------------------------------
# Host driver
# ---------------------------------------------------------------------------
_CACHE = {}


def _build():
    if "nc" in _CACHE:
        return _CACHE["nc"]
    nc = bacc.Bacc("TRN2", target_bir_lowering=False, debug=False, num_devices=NCORES)
    io = {}
    for name, shape, dts in IN_SPECS:
        io[name] = nc.dram_tensor(name, shape, BF16 if dts == "BF16" else F32, kind="ExternalInput").ap()
    io["out_dec"] = nc.dram_tensor("out_dec", [BL, STEPS, OUT], F32, kind="ExternalOutput").ap()
    with tile.TileContext(nc) as tc:
        decoder_kernel(tc, io)
    nc.compile()
    _CACHE["nc"] = nc
    return nc


def _chunked(w):
    """[k, j] -> [128, (kc, j)] with k = kc*128 + p."""
    k, j = w.shape
    return np.ascontiguousarray(w.reshape(k // 128, 128, j).transpose(1, 0, 2).reshape(128, -1))


def _prep_core(enc_l, h_l, attn_w, attn_b, w_ih, w_hh, b_ih, b_hh, fc_w, fc_b):
    wa_e, wa_d = attn_w[:H], attn_w[H:]
    enc_sb = np.ascontiguousarray(
        enc_l.reshape(BL, SO, 128, H).transpose(2, 0, 1, 3).reshape(128, -1)
    ).astype(BF)
    ee = enc_l @ wa_e + attn_b[0]
    enc_energy = np.ascontiguousarray(
        ee.reshape(BL, SO, 128).transpose(2, 0, 1).reshape(128, -1)
    ).astype(np.float32)

    w_d = w_ih[:, :OUT]                                   # dec-input part [2048, 256]
    w_c = w_ih[:, OUT:]                                   # ctx part [2048, 512]
    w_cmb = w_hh + w_d @ fc_w                             # [2048, 512]
    bias0 = b_ih + b_hh
    biasc = bias0 + w_d @ fc_b

    h0T = np.zeros((128, HC, 32), dtype=BF)
    h0T[:, :, :BL] = h_l.T.reshape(HC, 128, BL).transpose(1, 0, 2).astype(BF)
    return {
        "enc_sb": enc_sb,
        "enc_energy": enc_energy,
        "w_ihcT": _chunked(w_c.T).astype(BF),
        "w_hhT0": _chunked(w_hh.T).astype(BF),
        "w_cmbT": _chunked(w_cmb.T).astype(BF),
        "fc_wT": _chunked(fc_w.T).astype(BF),
        "wa_bc": np.ascontiguousarray(
            np.broadcast_to(wa_d.reshape(HC, 128, 1), (HC, 128, 128)).transpose(1, 0, 2).reshape(128, -1)
        ).astype(BF),
        "bias_g0": bias0.reshape(1, G4).astype(BF),
        "bias_gc": biasc.reshape(1, G4).astype(BF),
        "bias_fc": fc_b.reshape(1, OUT).astype(BF),
        "h0T": h0T.reshape(128, -1),
        "ident": np.eye(128, dtype=np.float32).astype(BF),
    }


def kernel(encoder_outputs, hidden, attn_w, attn_b, w_ih, w_hh, b_ih, b_hh, fc_w, fc_b):
    encoder_outputs = np.asarray(encoder_outputs, dtype=np.float32)
    hidden = np.asarray(hidden, dtype=np.float32)
    args = [np.asarray(a, dtype=np.float32) for a in (attn_w, attn_b, w_ih, w_hh, b_ih, b_hh, fc_w, fc_b)]

    nc = _build()
    in_maps = []
    for cidx in range(NCORES):
        sl = slice(cidx * BL, (cidx + 1) * BL)
        in_maps.append(_prep_core(encoder_outputs[sl], hidden[sl], *args))
    res = run_bass_kernel_spmd(nc, in_maps, list(range(NCORES)))
    outs = [res.results[cidx]["out_dec"] for cidx in range(NCORES)]
    return np.concatenate(outs, axis=0)
